# revision 15
# baseline (speedup 1.0000x reference)
"""DistogramLoss Trainium2 kernel (8-core SPMD, bass/tile).

Sharding: rows of the (b, i) pair-grid axis. Core c owns b = c//4 and
i in [192*(c%4), 192*(c%4)+192). The host rotates the j axis by -i0 so
the core's i-rows are always rows 0..192 of its inputs (the program is
SPMD-shared; j-reductions are order-invariant).

Layout: partitions = j (128 per block), free = (i, k) with 12 i's and
K=39 bins per supertile (free dim 468).
  L[j, 39*i+k] = sum_c V[j,c] * (wb[k,c]*U[i,c]) + bb[k]   (PE, bf16)
  ce = ln(sum_k exp(L)) - L[t]
Per supertile: one bf16 matmul with a 65th ones*bb row for the bias;
ACT exp (bf16 out); DVE grouped-reduce for sum_k exp; one-hot mask via
GPSIMD broadcast-copy + DVE bf16 is_equal; sum of L[target] via
scalar_tensor_tensor(mask*L) accum_out into a per-supertile column.
All ln's are batched into a single end-of-kernel ACT op (avoids
per-supertile activation-table reloads).
"""

import os
import sys

for _p in ("/opt/trn_rl_repo", "/opt/pypackages"):
    if os.path.isdir(_p) and _p not in sys.path:
        sys.path.append(_p)

import numpy as np

import concourse.bacc as bacc
import concourse.bass as bass
import concourse.tile as tile
from concourse import mybir
from concourse.bass_utils import run_bass_kernel_spmd

F32 = mybir.dt.float32
BF16 = mybir.dt.bfloat16
AX = mybir.AxisListType
ALU = mybir.AluOpType
ACTF = mybir.ActivationFunctionType

B, N, D, DL, K = 2, 768, 512, 64, 39
DIST_MIN, DIST_MAX = 2.0, 22.0
W = (DIST_MAX - DIST_MIN) / (K - 1)
LN_EPS = 1e-5

NCORES = 8
NI = (B * N) // NCORES          # 192 i-rows per core
IB = 12                          # i's per supertile
NIB = NI // IB                   # 16 supertiles along i
JB = 128                         # j's per block (partitions)
NJB = N // JB                    # 6 j blocks
FD = IB * K                      # 468 free dim of a supertile
NST = NJB * NIB                  # 96 supertiles
POISON = 3.0 * K                 # target offset that can never match k


def _bcast_free(ap, reps):
    """Append a 0-step dim of size `reps` to an AP (free-dim broadcast)."""
    return bass.AP(tensor=ap.tensor, offset=ap.offset, ap=list(ap.ap) + [[0, reps]])


def _build_program(with_poison: bool):
    nc = bacc.Bacc("TRN2", target_bir_lowering=False, debug=False)

    h_rows = nc.dram_tensor("h_rows", [N, D], F32, kind="ExternalInput")
    dl5 = nc.dram_tensor("dl5", [5, N], F32, kind="ExternalInput")
    dr5 = nc.dram_tensor("dr5", [5, NI], F32, kind="ExternalInput")
    wt_uv = nc.dram_tensor("wt_uv", [128, 4, 128], F32, kind="ExternalInput")
    uvb = nc.dram_tensor("uvb", [128, 1], F32, kind="ExternalInput")
    wb_rep = nc.dram_tensor("wb_rep", [DL, FD], F32, kind="ExternalInput")
    bb_rep = nc.dram_tensor("bb_rep", [1, FD], BF16, kind="ExternalInput")
    krow_row = nc.dram_tensor("krow_row", [1, FD], F32, kind="ExternalInput")
    mj_cols = nc.dram_tensor("mj_cols", [JB, NJB], F32, kind="ExternalInput")
    ident = nc.dram_tensor("ident", [128, 128], F32, kind="ExternalInput")
    if with_poison:
        poisj_cols = nc.dram_tensor("poisj_cols", [JB, NJB], F32, kind="ExternalInput")
        pois_i = nc.dram_tensor("pois_i", [1, NI], F32, kind="ExternalInput")

    out_lse = nc.dram_tensor("out_lse", [JB, NI], F32, kind="ExternalOutput")
    out_ext = nc.dram_tensor("out_ext", [JB, NST], F32, kind="ExternalOutput")

    with tile.TileContext(nc) as tc:
        with (
            tc.tile_pool(name="const", bufs=1) as const,
            tc.tile_pool(name="work", bufs=3) as work,
            tc.tile_pool(name="small", bufs=6) as small,
            tc.tile_pool(name="ebuf", bufs=3) as ebuf,
            tc.tile_pool(name="mbuf", bufs=3) as mbuf,
            tc.tile_pool(name="tbuf", bufs=3) as tbuf,
            tc.tile_pool(name="jbuf", bufs=2) as jbuf,
            tc.tile_pool(name="pp", bufs=2, space="PSUM") as pp,
            tc.tile_pool(name="psl", bufs=4, space="PSUM") as psl,
        ):
            # ---------------- constants into SBUF ----------------
            sb_wtuv = const.tile([128, 4, 128], F32)
            nc.sync.dma_start(out=sb_wtuv[:], in_=wt_uv[:])
            sb_uvb = const.tile([128, 1], F32)
            nc.sync.dma_start(out=sb_uvb[:], in_=uvb[:])
            sb_wbrep = const.tile([DL, FD], F32)
            nc.sync.dma_start(out=sb_wbrep[:], in_=wb_rep[:])
            sb_dl = const.tile([5, N], F32)
            nc.sync.dma_start(out=sb_dl[:], in_=dl5[:])
            sb_dr = const.tile([5, NI], F32)
            nc.sync.dma_start(out=sb_dr[:], in_=dr5[:])
            sb_mj = const.tile([JB, NJB], F32)
            nc.sync.dma_start(out=sb_mj[:], in_=mj_cols[:])
            sb_ident = const.tile([128, 128], F32)
            nc.sync.dma_start(out=sb_ident[:], in_=ident[:])
            sb_krow = const.tile([128, FD], F32)
            nc.sync.dma_start(
                out=sb_krow[:],
                in_=bass.AP(tensor=krow_row, offset=0, ap=[[0, 128], [1, FD]]),
            )
            if with_poison:
                sb_poisj = const.tile([JB, NJB], F32)
                nc.sync.dma_start(out=sb_poisj[:], in_=poisj_cols[:])
                sb_poisi = const.tile([1, NI], F32)
                nc.sync.dma_start(out=sb_poisi[:], in_=pois_i[:])

            sb_eps = const.tile([128, 1], F32)
            nc.vector.memset(sb_eps[:], LN_EPS)

            s_all = const.tile([JB, NJB, NI], F32)      # sum_k exp, per (jb, i)
            ext_all = const.tile([JB, NST], F32)        # sum mask*L per supertile
            acc_lse = const.tile([JB, NI], F32)
            nc.vector.memset(acc_lse[:], 0.0)

            # ---------------- LN + transpose + projections ----------------
            hT = const.tile([128, 4, N], F32)  # h^T, c-chunk q on partitions
            for blk in range(NJB):
                hb = work.tile([128, D], F32, tag="hb")
                nc.sync.dma_start(out=hb[:], in_=h_rows[blk * 128:(blk + 1) * 128, :])
                stats = small.tile([128, 6], F32, tag="stats")
                nc.vector.bn_stats(out=stats[:], in_=hb[:])
                mv = small.tile([128, 2], F32, tag="mv")
                nc.vector.bn_aggr(out=mv[:], in_=stats[:])
                std = small.tile([128, 1], F32, tag="std")
                nc.scalar.activation(std[:], mv[:, 1:2], ACTF.Sqrt, bias=sb_eps[:, 0:1])
                rstd = small.tile([128, 1], F32, tag="rstd")
                nc.vector.reciprocal(rstd[:], std[:])
                nb = small.tile([128, 1], F32, tag="nb")
                nc.vector.tensor_scalar(
                    out=nb[:], in0=mv[:, 0:1], scalar1=rstd[:, 0:1], scalar2=-1.0,
                    op0=ALU.mult, op1=ALU.mult,
                )
                hn = work.tile([128, D], F32, tag="hn")
                nc.scalar.activation(
                    hn[:], hb[:], ACTF.Identity, bias=nb[:, 0:1], scale=rstd[:, 0:1],
                )
                for q in range(4):
                    pt = pp.tile([128, 128], F32, tag="pp")
                    nc.tensor.transpose(pt[:], hn[:, q * 128:(q + 1) * 128], sb_ident[:])
                    nc.vector.tensor_copy(hT[:, q, blk * 128:(blk + 1) * 128], pt[:])

            uv = const.tile([128, N], F32)  # rows 0:64 U^T, 64:128 V^T
            for half in range(2):
                pu = pp.tile([128, N // 2], F32, tag="pp")
                for q in range(4):
                    nc.tensor.matmul(
                        out=pu[:], lhsT=sb_wtuv[:, q, :],
                        rhs=hT[:, q, half * (N // 2):(half + 1) * (N // 2)],
                        start=(q == 0), stop=(q == 3),
                    )
                nc.scalar.activation(
                    uv[:, half * (N // 2):(half + 1) * (N // 2)], pu[:],
                    ACTF.Identity, bias=sb_uvb[:, 0:1],
                )
            # V^T (bf16) at partitions 0:64 + ones row 64 (pairs with bb row)
            vtf = const.tile([DL, N], F32)
            nc.sync.dma_start(out=vtf[:], in_=uv[DL:128, :])
            vt65 = const.tile([DL + 1, N], BF16)
            nc.vector.tensor_copy(vt65[0:DL, :], vtf[:])
            nc.vector.memset(vt65[DL:DL + 1, :], 1.0)

            # ---------------- targets T[j, i] per j-block (bf16) ----------
            t_all = const.tile([128, NJB, NI], F32)
            for jb in range(NJB):
                pd = pp.tile([128, NI], F32, tag="pp")
                nc.tensor.matmul(
                    out=pd[:], lhsT=sb_dl[:, jb * 128:(jb + 1) * 128], rhs=sb_dr[:],
                    start=True, stop=True,
                )
                dsq = work.tile([128, NI], F32, tag="dsq")
                nc.vector.tensor_scalar(
                    out=dsq[:], in0=pd[:], scalar1=0.0, scalar2=None, op0=ALU.max,
                )
                yv = work.tile([128, NI], F32, tag="yv")  # sqrt(dsq)/W
                nc.scalar.activation(yv[:], dsq[:], ACTF.Sqrt, scale=1.0 / (W * W))
                y = work.tile([128, NI], F32, tag="y")  # (d - 2)/W
                nc.vector.tensor_scalar(
                    out=y[:], in0=yv[:], scalar1=DIST_MIN / W, scalar2=None,
                    op0=ALU.subtract,
                )
                ti = work.tile([128, NI], mybir.dt.int32, tag="ti")
                nc.vector.tensor_copy(ti[:], y[:])
                tf = work.tile([128, NI], F32, tag="tf")
                nc.vector.tensor_copy(tf[:], ti[:])
                gt = work.tile([128, NI], F32, tag="gt")
                nc.vector.tensor_tensor(out=gt[:], in0=tf[:], in1=y[:], op=ALU.is_gt)
                t0 = work.tile([128, NI], F32, tag="t0")
                nc.vector.tensor_tensor(out=t0[:], in0=tf[:], in1=gt[:], op=ALU.subtract)
                if with_poison:
                    t1 = work.tile([128, NI], F32, tag="t1")
                    nc.vector.tensor_scalar(
                        out=t1[:], in0=t0[:], scalar1=0.0, scalar2=float(K - 1),
                        op0=ALU.max, op1=ALU.min,
                    )
                    t2 = work.tile([128, NI], F32, tag="t2")
                    nc.vector.tensor_scalar(
                        out=t2[:], in0=t1[:], scalar1=sb_poisj[:, jb:jb + 1],
                        scalar2=None, op0=ALU.add,
                    )
                    pi = pp.tile([128, NI], F32, tag="pp")
                    oner = small.tile([1, 128], F32, tag="oner")
                    nc.vector.memset(oner[:], 1.0)
                    nc.tensor.matmul(
                        out=pi[:], lhsT=oner[:], rhs=sb_poisi[:],
                        start=True, stop=True,
                    )
                    nc.vector.tensor_tensor(
                        out=t_all[:, jb, :], in0=t2[:], in1=pi[:], op=ALU.add,
                    )
                else:
                    nc.vector.tensor_scalar(
                        out=t_all[:, jb, :], in0=t0[:], scalar1=0.0,
                        scalar2=float(K - 1), op0=ALU.max, op1=ALU.min,
                    )

            # -------- WU65[c, (i,k)] = wb[k,c]*U[i,c]; row 64 = bb ---------
            wu65 = const.tile([DL + 1, NIB, FD], BF16)
            wb3 = sb_wbrep[:].rearrange("p (i k) -> p i k", k=K)
            for ib in range(NIB):
                u_sl = uv[0:DL, ib * IB:(ib + 1) * IB]
                nc.vector.tensor_tensor(
                    out=wu65[0:DL, ib, :].rearrange("p (i k) -> p i k", k=K),
                    in0=wb3, in1=_bcast_free(u_sl, K), op=ALU.mult,
                )
            nc.sync.dma_start(
                out=wu65[DL:DL + 1, :, :],
                in_=bass.AP(tensor=bb_rep, offset=0, ap=[[0, 1], [0, NIB], [1, FD]]),
            )

            # ---------------- main loop ----------------
            for jb in range(NJB):
                for ib in range(NIB):
                    st = jb * NIB + ib
                    pl = psl.tile([128, FD], F32, tag="psl")
                    nc.tensor.matmul(
                        out=pl[:], lhsT=vt65[:, jb * 128:(jb + 1) * 128],
                        rhs=wu65[:, ib, :], start=True, stop=True,
                    )
                    if st % 2 == 1:
                        # ACT-mode: per-i exp with free-dim accumulate; the
                        # sum_k exp comes from accum_out, no DVE reduce.
                        ej = ebuf.tile([128, K], BF16, tag="ej")
                        for ii in range(IB):
                            nc.scalar.activation(
                                ej[:], pl[:, ib * 0 + ii * K:(ii + 1) * K],
                                ACTF.Exp,
                                accum_out=s_all[:, jb, ib * IB + ii:ib * IB + ii + 1],
                            )
                    else:
                        e = ebuf.tile([128, FD], BF16, tag="e")
                        nc.scalar.activation(e[:], pl[:], ACTF.Exp)
                        nc.vector.reduce_sum(
                            out=s_all[:, jb, ib * IB:(ib + 1) * IB],
                            in_=e[:].rearrange("p (i k) -> p i k", k=K), axis=AX.X,
                        )
                    t_sl = t_all[:, jb, ib * IB:(ib + 1) * IB]
                    msk = mbuf.tile([128, FD], BF16, tag="msk")
                    nc.vector.tensor_tensor(
                        out=msk[:].rearrange("p (i k) -> p i k", k=K),
                        in0=_bcast_free(t_sl, K),
                        in1=sb_krow[:].rearrange("p (i k) -> p i k", k=K),
                        op=ALU.is_equal,
                    )
                    junk = jbuf.tile([128, FD], BF16, tag="junk")
                    nc.vector.scalar_tensor_tensor(
                        out=junk[:], in0=msk[:], scalar=1.0, in1=pl[:],
                        op0=ALU.mult, op1=ALU.mult,
                        accum_out=ext_all[:, st:st + 1],
                    )

            # ---------------- epilogue: batched ln + masked sums ----------
            lse_all = const.tile([JB, NJB, NI], F32)
            nc.scalar.activation(lse_all[:], s_all[:], ACTF.Ln)
            for jb in range(NJB):
                nc.vector.scalar_tensor_tensor(
                    out=acc_lse[:], in0=lse_all[:, jb, :],
                    scalar=sb_mj[:, jb:jb + 1], in1=acc_lse[:],
                    op0=ALU.mult, op1=ALU.add,
                )

            nc.sync.dma_start(out=out_lse[:], in_=acc_lse[:])
            nc.sync.dma_start(out=out_ext[:], in_=ext_all[:])

    nc.finalize()
    return nc


_PROGRAM_CACHE: dict = {}


def _get_program(with_poison: bool):
    if with_poison not in _PROGRAM_CACHE:
        _PROGRAM_CACHE[with_poison] = _build_program(with_poison)
    return _PROGRAM_CACHE[with_poison]


def _prep_core_inputs(core, h_res, x_true, token_pad_mask, shared, with_poison):
    # The device program is SPMD-shared, so the U-projection always reads
    # rows 0..NI. Rotate the whole j-axis by -i0 on the host so the core's
    # i-slice lands at rows 0..NI; every j-reduction is order-invariant.
    b = core // (NCORES // B)
    i0 = NI * (core % (NCORES // B))
    x = np.roll(np.asarray(x_true[b], np.float32), -i0, axis=0)      # [N, 3]
    n2 = (x * x).sum(-1).astype(np.float32)                          # [N]
    m = np.roll(np.asarray(token_pad_mask[b], np.float32), -i0)      # [N]

    dl = np.empty((5, N), np.float32)
    dl[0:3] = -2.0 * x.T
    dl[3] = 1.0
    dl[4] = n2
    dr = np.empty((5, NI), np.float32)
    dr[0:3] = x.T[:, :NI]
    dr[3] = n2[:NI]
    dr[4] = 1.0

    inp = dict(shared)
    inp["h_rows"] = np.ascontiguousarray(
        np.roll(np.asarray(h_res[b], np.float32), -i0, axis=0))
    inp["dl5"] = dl
    inp["dr5"] = dr
    inp["mj_cols"] = np.ascontiguousarray(m.reshape(NJB, JB).T)
    if with_poison:
        inp["poisj_cols"] = np.ascontiguousarray(
            (POISON * (1.0 - m)).reshape(NJB, JB).T.astype(np.float32))
        inp["pois_i"] = (POISON * (1.0 - m[:NI]))[None, :].astype(np.float32)
    return inp


def _host_finish(results, token_pad_mask):
    mask = np.asarray(token_pad_mask, np.float64)
    ce_b = np.zeros(B, np.float64)
    per_b = NCORES // B
    for core, res in enumerate(results):
        b = core // per_b
        i0 = NI * (core % per_b)
        m_i = mask[b, i0:i0 + NI]
        lse_i = np.asarray(res["out_lse"], np.float64).sum(axis=0)  # [NI]
        ce_b[b] += float((m_i * lse_i).sum()) - float(
            np.asarray(res["out_ext"], np.float64).sum())
    counts = mask.sum(axis=1) ** 2
    per_sample = ce_b / np.maximum(counts, 1.0)
    valid = counts > 0
    total = max(float(valid.sum()), 1.0)
    loss = float(np.where(valid, per_sample, 0.0).sum() / total)
    return np.float32(loss)


def _shared_inputs(ln_w, ln_b, wu_w, wu_b, wv_w, wv_b, wb_w, wb_b):
    import ml_dtypes
    bf = ml_dtypes.bfloat16
    ln_w = np.asarray(ln_w, np.float32)
    ln_b = np.asarray(ln_b, np.float32)
    wu2 = np.asarray(wu_w, np.float32) * ln_w[None, :]
    wv2 = np.asarray(wv_w, np.float32) * ln_w[None, :]
    wub2 = np.asarray(wu_b, np.float32) + np.asarray(wu_w, np.float32) @ ln_b
    wvb2 = np.asarray(wv_b, np.float32) + np.asarray(wv_w, np.float32) @ ln_b

    wt = np.concatenate([wu2.T, wv2.T], axis=1)  # [512, 128]
    wt_uv = np.ascontiguousarray(wt.reshape(4, 128, 128).transpose(1, 0, 2))
    uvb = np.concatenate([wub2, wvb2])[:, None].astype(np.float32)

    wb_rep = np.ascontiguousarray(
        np.tile(np.asarray(wb_w, np.float32).T, (1, IB)))          # [64, 468]
    bb_rep = np.ascontiguousarray(
        np.tile(np.asarray(wb_b, np.float32), IB))[None, :].astype(bf)
    krow_row = np.tile(np.arange(K, dtype=np.float32), IB)[None, :]
    ident = np.eye(128, dtype=np.float32)
    return {
        "wt_uv": wt_uv, "uvb": uvb, "wb_rep": wb_rep, "bb_rep": bb_rep,
        "krow_row": krow_row, "ident": ident,
    }


def kernel(h_res, x_true, token_pad_mask, ln_w, ln_b, wu_w, wu_b, wv_w, wv_b,
           wb_w, wb_b):
    mask_np = np.asarray(token_pad_mask, np.float32)
    with_poison = not bool(np.all(mask_np == 1.0))
    nc = _get_program(with_poison)
    shared = _shared_inputs(ln_w, ln_b, wu_w, wu_b, wv_w, wv_b, wb_w, wb_b)
    in_maps = [
        _prep_core_inputs(c, h_res, x_true, mask_np, shared, with_poison)
        for c in range(NCORES)
    ]
    res = run_bass_kernel_spmd(nc, in_maps, core_ids=list(range(NCORES)))
    return _host_finish(res.results, mask_np)


# revision 16
# speedup vs baseline: 1.3676x; 1.3676x over previous
"""DistogramLoss Trainium2 kernel (8-core SPMD, bass/tile).

Sharding: rows of the (b, i) pair-grid axis. Core c owns b = c//4 and
i in [192*(c%4), 192*(c%4)+192). The host rotates the j axis by -i0 so
the core's i-rows are always rows 0..192 of its inputs (the program is
SPMD-shared; j-reductions are order-invariant).

Layout: partitions = j (128 per block), free = (i, k) with 12 i's and
K=39 bins per supertile (free dim 468).
  L[j, 39*i+k] = sum_c V[j,c] * (wb[k,c]*U[i,c]) + bb[k]   (PE, bf16)
  ce = ln(sum_k exp(L)) - L[t]
Per supertile: one bf16 matmul with a 65th ones*bb row for the bias;
ACT exp (bf16 out); DVE grouped-reduce for sum_k exp; one-hot mask via
GPSIMD broadcast-copy + DVE bf16 is_equal; sum of L[target] via
scalar_tensor_tensor(mask*L) accum_out into a per-supertile column.
All ln's are batched into a single end-of-kernel ACT op (avoids
per-supertile activation-table reloads).
"""

import os
import sys

for _p in ("/opt/trn_rl_repo", "/opt/pypackages"):
    if os.path.isdir(_p) and _p not in sys.path:
        sys.path.append(_p)

import numpy as np

import concourse.bacc as bacc
import concourse.bass as bass
import concourse.tile as tile
from concourse import mybir
from concourse.bass_utils import run_bass_kernel_spmd

F32 = mybir.dt.float32
BF16 = mybir.dt.bfloat16
AX = mybir.AxisListType
ALU = mybir.AluOpType
ACTF = mybir.ActivationFunctionType

B, N, D, DL, K = 2, 768, 512, 64, 39
DIST_MIN, DIST_MAX = 2.0, 22.0
W = (DIST_MAX - DIST_MIN) / (K - 1)
LN_EPS = 1e-5

NCORES = 8
NI = (B * N) // NCORES          # 192 i-rows per core
IB = 12                          # i's per supertile
NIB = NI // IB                   # 16 supertiles along i
JB = 128                         # j's per block (partitions)
NJB = N // JB                    # 6 j blocks
FD = IB * K                      # 468 free dim of a supertile
NST = NJB * NIB                  # 96 supertiles
POISON = 3.0 * K                 # target offset that can never match k


def _bcast_free(ap, reps):
    """Append a 0-step dim of size `reps` to an AP (free-dim broadcast)."""
    return bass.AP(tensor=ap.tensor, offset=ap.offset, ap=list(ap.ap) + [[0, reps]])


def _build_program(with_poison: bool):
    nc = bacc.Bacc("TRN2", target_bir_lowering=False, debug=False)

    h_rows = nc.dram_tensor("h_rows", [N, D], F32, kind="ExternalInput")
    dl5 = nc.dram_tensor("dl5", [5, N], F32, kind="ExternalInput")
    dr5 = nc.dram_tensor("dr5", [5, NI], F32, kind="ExternalInput")
    wt_uv = nc.dram_tensor("wt_uv", [128, 4, 128], F32, kind="ExternalInput")
    uvb = nc.dram_tensor("uvb", [128, 1], F32, kind="ExternalInput")
    wb_rep = nc.dram_tensor("wb_rep", [DL, FD], F32, kind="ExternalInput")
    bb_rep = nc.dram_tensor("bb_rep", [1, FD], BF16, kind="ExternalInput")
    krow_row = nc.dram_tensor("krow_row", [1, FD], F32, kind="ExternalInput")
    mj_cols = nc.dram_tensor("mj_cols", [JB, NJB], F32, kind="ExternalInput")
    ident = nc.dram_tensor("ident", [128, 128], F32, kind="ExternalInput")
    if with_poison:
        poisj_cols = nc.dram_tensor("poisj_cols", [JB, NJB], F32, kind="ExternalInput")
        pois_i = nc.dram_tensor("pois_i", [1, NI], F32, kind="ExternalInput")

    out_lse = nc.dram_tensor("out_lse", [JB, NI], F32, kind="ExternalOutput")
    out_ext = nc.dram_tensor("out_ext", [JB, NST], F32, kind="ExternalOutput")

    with tile.TileContext(nc) as tc:
        with (
            tc.tile_pool(name="const", bufs=1) as const,
            tc.tile_pool(name="work", bufs=3) as work,
            tc.tile_pool(name="small", bufs=6) as small,
            tc.tile_pool(name="ebuf", bufs=4) as ebuf,
            tc.tile_pool(name="mbuf", bufs=4) as mbuf,
            tc.tile_pool(name="tbuf", bufs=3) as tbuf,
            tc.tile_pool(name="jbuf", bufs=2) as jbuf,
            tc.tile_pool(name="pp", bufs=2, space="PSUM") as pp,
            tc.tile_pool(name="psl", bufs=5, space="PSUM") as psl,
        ):
            # ---------------- constants into SBUF ----------------
            sb_wtuv = const.tile([128, 4, 128], F32)
            nc.sync.dma_start(out=sb_wtuv[:], in_=wt_uv[:])
            sb_uvb = const.tile([128, 1], F32)
            nc.sync.dma_start(out=sb_uvb[:], in_=uvb[:])
            sb_wbrep = const.tile([DL, FD], F32)
            nc.sync.dma_start(out=sb_wbrep[:], in_=wb_rep[:])
            sb_dl = const.tile([5, N], F32)
            nc.sync.dma_start(out=sb_dl[:], in_=dl5[:])
            sb_dr = const.tile([5, NI], F32)
            nc.sync.dma_start(out=sb_dr[:], in_=dr5[:])
            sb_mj = const.tile([JB, NJB], F32)
            nc.sync.dma_start(out=sb_mj[:], in_=mj_cols[:])
            sb_ident = const.tile([128, 128], F32)
            nc.sync.dma_start(out=sb_ident[:], in_=ident[:])
            sb_krow = const.tile([128, FD], F32)
            nc.sync.dma_start(
                out=sb_krow[:],
                in_=bass.AP(tensor=krow_row, offset=0, ap=[[0, 128], [1, FD]]),
            )
            if with_poison:
                sb_poisj = const.tile([JB, NJB], F32)
                nc.sync.dma_start(out=sb_poisj[:], in_=poisj_cols[:])
                sb_poisi = const.tile([1, NI], F32)
                nc.sync.dma_start(out=sb_poisi[:], in_=pois_i[:])

            sb_eps = const.tile([128, 1], F32)
            nc.vector.memset(sb_eps[:], LN_EPS)

            s_all = const.tile([JB, NJB, NI], F32)      # sum_k exp, per (jb, i)
            ext_all = const.tile([JB, NST], F32)        # sum mask*L per supertile
            acc_lse = const.tile([JB, NI], F32)
            nc.vector.memset(acc_lse[:], 0.0)

            # ---------------- LN + transpose + projections ----------------
            hT = const.tile([128, 4, N], F32)  # h^T, c-chunk q on partitions
            for blk in range(NJB):
                hb = work.tile([128, D], F32, tag="hb")
                nc.sync.dma_start(out=hb[:], in_=h_rows[blk * 128:(blk + 1) * 128, :])
                stats = small.tile([128, 6], F32, tag="stats")
                nc.vector.bn_stats(out=stats[:], in_=hb[:])
                mv = small.tile([128, 2], F32, tag="mv")
                nc.vector.bn_aggr(out=mv[:], in_=stats[:])
                std = small.tile([128, 1], F32, tag="std")
                nc.scalar.activation(std[:], mv[:, 1:2], ACTF.Sqrt, bias=sb_eps[:, 0:1])
                rstd = small.tile([128, 1], F32, tag="rstd")
                nc.vector.reciprocal(rstd[:], std[:])
                nb = small.tile([128, 1], F32, tag="nb")
                nc.vector.tensor_scalar(
                    out=nb[:], in0=mv[:, 0:1], scalar1=rstd[:, 0:1], scalar2=-1.0,
                    op0=ALU.mult, op1=ALU.mult,
                )
                hn = work.tile([128, D], F32, tag="hn")
                nc.scalar.activation(
                    hn[:], hb[:], ACTF.Identity, bias=nb[:, 0:1], scale=rstd[:, 0:1],
                )
                for q in range(4):
                    pt = pp.tile([128, 128], F32, tag="pp")
                    nc.tensor.transpose(pt[:], hn[:, q * 128:(q + 1) * 128], sb_ident[:])
                    nc.vector.tensor_copy(hT[:, q, blk * 128:(blk + 1) * 128], pt[:])

            uv = const.tile([128, N], F32)  # rows 0:64 U^T, 64:128 V^T
            for half in range(2):
                pu = pp.tile([128, N // 2], F32, tag="pp")
                for q in range(4):
                    nc.tensor.matmul(
                        out=pu[:], lhsT=sb_wtuv[:, q, :],
                        rhs=hT[:, q, half * (N // 2):(half + 1) * (N // 2)],
                        start=(q == 0), stop=(q == 3),
                    )
                nc.scalar.activation(
                    uv[:, half * (N // 2):(half + 1) * (N // 2)], pu[:],
                    ACTF.Identity, bias=sb_uvb[:, 0:1],
                )
            # V^T (bf16) at partitions 0:64 + ones row 64 (pairs with bb row)
            vtf = const.tile([DL, N], F32)
            nc.sync.dma_start(out=vtf[:], in_=uv[DL:128, :])
            vt65 = const.tile([DL + 1, N], BF16)
            nc.vector.tensor_copy(vt65[0:DL, :], vtf[:])
            nc.vector.memset(vt65[DL:DL + 1, :], 1.0)

            # ---------------- targets T[j, i] per j-block (bf16) ----------
            t_all = const.tile([128, NJB, NI], F32)
            for jb in range(NJB):
                pd = pp.tile([128, NI], F32, tag="pp")
                nc.tensor.matmul(
                    out=pd[:], lhsT=sb_dl[:, jb * 128:(jb + 1) * 128], rhs=sb_dr[:],
                    start=True, stop=True,
                )
                dsq = work.tile([128, NI], F32, tag="dsq")
                nc.vector.tensor_scalar(
                    out=dsq[:], in0=pd[:], scalar1=0.0, scalar2=None, op0=ALU.max,
                )
                yv = work.tile([128, NI], F32, tag="yv")  # sqrt(dsq)/W
                nc.scalar.activation(yv[:], dsq[:], ACTF.Sqrt, scale=1.0 / (W * W))
                y = work.tile([128, NI], F32, tag="y")  # (d - 2)/W
                nc.vector.tensor_scalar(
                    out=y[:], in0=yv[:], scalar1=DIST_MIN / W, scalar2=None,
                    op0=ALU.subtract,
                )
                ti = work.tile([128, NI], mybir.dt.int32, tag="ti")
                nc.vector.tensor_copy(ti[:], y[:])
                tf = work.tile([128, NI], F32, tag="tf")
                nc.vector.tensor_copy(tf[:], ti[:])
                gt = work.tile([128, NI], F32, tag="gt")
                nc.vector.tensor_tensor(out=gt[:], in0=tf[:], in1=y[:], op=ALU.is_gt)
                t0 = work.tile([128, NI], F32, tag="t0")
                nc.vector.tensor_tensor(out=t0[:], in0=tf[:], in1=gt[:], op=ALU.subtract)
                if with_poison:
                    t1 = work.tile([128, NI], F32, tag="t1")
                    nc.vector.tensor_scalar(
                        out=t1[:], in0=t0[:], scalar1=0.0, scalar2=float(K - 1),
                        op0=ALU.max, op1=ALU.min,
                    )
                    t2 = work.tile([128, NI], F32, tag="t2")
                    nc.vector.tensor_scalar(
                        out=t2[:], in0=t1[:], scalar1=sb_poisj[:, jb:jb + 1],
                        scalar2=None, op0=ALU.add,
                    )
                    pi = pp.tile([128, NI], F32, tag="pp")
                    oner = small.tile([1, 128], F32, tag="oner")
                    nc.vector.memset(oner[:], 1.0)
                    nc.tensor.matmul(
                        out=pi[:], lhsT=oner[:], rhs=sb_poisi[:],
                        start=True, stop=True,
                    )
                    nc.vector.tensor_tensor(
                        out=t_all[:, jb, :], in0=t2[:], in1=pi[:], op=ALU.add,
                    )
                else:
                    nc.vector.tensor_scalar(
                        out=t_all[:, jb, :], in0=t0[:], scalar1=0.0,
                        scalar2=float(K - 1), op0=ALU.max, op1=ALU.min,
                    )

            # -------- WU65[c, (i,k)] = wb[k,c]*U[i,c]; row 64 = bb ---------
            wu65 = const.tile([DL + 1, NIB, FD], BF16)
            wb3 = sb_wbrep[:].rearrange("p (i k) -> p i k", k=K)
            for ib in range(NIB):
                u_sl = uv[0:DL, ib * IB:(ib + 1) * IB]
                nc.vector.tensor_tensor(
                    out=wu65[0:DL, ib, :].rearrange("p (i k) -> p i k", k=K),
                    in0=wb3, in1=_bcast_free(u_sl, K), op=ALU.mult,
                )
            nc.sync.dma_start(
                out=wu65[DL:DL + 1, :, :],
                in_=bass.AP(tensor=bb_rep, offset=0, ap=[[0, 1], [0, NIB], [1, FD]]),
            )

            # ---------------- main loop ----------------
            for jb in range(NJB):
                for ib in range(NIB):
                    st = jb * NIB + ib
                    pl = psl.tile([128, FD], F32, tag="psl")
                    nc.tensor.matmul(
                        out=pl[:], lhsT=vt65[:, jb * 128:(jb + 1) * 128],
                        rhs=wu65[:, ib, :], start=True, stop=True,
                    )
                    e = ebuf.tile([128, FD], BF16, tag="e")
                    nc.scalar.activation(e[:], pl[:], ACTF.Exp)
                    nc.vector.reduce_sum(
                        out=s_all[:, jb, ib * IB:(ib + 1) * IB],
                        in_=e[:].rearrange("p (i k) -> p i k", k=K), axis=AX.X,
                    )
                    t_sl = t_all[:, jb, ib * IB:(ib + 1) * IB]
                    msk = mbuf.tile([128, FD], BF16, tag="msk")
                    nc.vector.tensor_tensor(
                        out=msk[:].rearrange("p (i k) -> p i k", k=K),
                        in0=_bcast_free(t_sl, K),
                        in1=sb_krow[:].rearrange("p (i k) -> p i k", k=K),
                        op=ALU.is_equal,
                    )
                    junk = jbuf.tile([128, FD], BF16, tag="junk")
                    nc.vector.scalar_tensor_tensor(
                        out=junk[:], in0=msk[:], scalar=1.0, in1=pl[:],
                        op0=ALU.mult, op1=ALU.mult,
                        accum_out=ext_all[:, st:st + 1],
                    )

            # ---------------- epilogue: batched ln + masked sums ----------
            lse_all = const.tile([JB, NJB, NI], F32)
            nc.scalar.activation(lse_all[:], s_all[:], ACTF.Ln)
            for jb in range(NJB):
                nc.vector.scalar_tensor_tensor(
                    out=acc_lse[:], in0=lse_all[:, jb, :],
                    scalar=sb_mj[:, jb:jb + 1], in1=acc_lse[:],
                    op0=ALU.mult, op1=ALU.add,
                )

            nc.sync.dma_start(out=out_lse[:], in_=acc_lse[:])
            nc.sync.dma_start(out=out_ext[:], in_=ext_all[:])

    nc.finalize()
    return nc


_PROGRAM_CACHE: dict = {}


def _get_program(with_poison: bool):
    if with_poison not in _PROGRAM_CACHE:
        _PROGRAM_CACHE[with_poison] = _build_program(with_poison)
    return _PROGRAM_CACHE[with_poison]


def _prep_core_inputs(core, h_res, x_true, token_pad_mask, shared, with_poison):
    # The device program is SPMD-shared, so the U-projection always reads
    # rows 0..NI. Rotate the whole j-axis by -i0 on the host so the core's
    # i-slice lands at rows 0..NI; every j-reduction is order-invariant.
    b = core // (NCORES // B)
    i0 = NI * (core % (NCORES // B))
    x = np.roll(np.asarray(x_true[b], np.float32), -i0, axis=0)      # [N, 3]
    n2 = (x * x).sum(-1).astype(np.float32)                          # [N]
    m = np.roll(np.asarray(token_pad_mask[b], np.float32), -i0)      # [N]

    dl = np.empty((5, N), np.float32)
    dl[0:3] = -2.0 * x.T
    dl[3] = 1.0
    dl[4] = n2
    dr = np.empty((5, NI), np.float32)
    dr[0:3] = x.T[:, :NI]
    dr[3] = n2[:NI]
    dr[4] = 1.0

    inp = dict(shared)
    inp["h_rows"] = np.ascontiguousarray(
        np.roll(np.asarray(h_res[b], np.float32), -i0, axis=0))
    inp["dl5"] = dl
    inp["dr5"] = dr
    inp["mj_cols"] = np.ascontiguousarray(m.reshape(NJB, JB).T)
    if with_poison:
        inp["poisj_cols"] = np.ascontiguousarray(
            (POISON * (1.0 - m)).reshape(NJB, JB).T.astype(np.float32))
        inp["pois_i"] = (POISON * (1.0 - m[:NI]))[None, :].astype(np.float32)
    return inp


def _host_finish(results, token_pad_mask):
    mask = np.asarray(token_pad_mask, np.float64)
    ce_b = np.zeros(B, np.float64)
    per_b = NCORES // B
    for core, res in enumerate(results):
        b = core // per_b
        i0 = NI * (core % per_b)
        m_i = mask[b, i0:i0 + NI]
        lse_i = np.asarray(res["out_lse"], np.float64).sum(axis=0)  # [NI]
        ce_b[b] += float((m_i * lse_i).sum()) - float(
            np.asarray(res["out_ext"], np.float64).sum())
    counts = mask.sum(axis=1) ** 2
    per_sample = ce_b / np.maximum(counts, 1.0)
    valid = counts > 0
    total = max(float(valid.sum()), 1.0)
    loss = float(np.where(valid, per_sample, 0.0).sum() / total)
    return np.float32(loss)


def _shared_inputs(ln_w, ln_b, wu_w, wu_b, wv_w, wv_b, wb_w, wb_b):
    import ml_dtypes
    bf = ml_dtypes.bfloat16
    ln_w = np.asarray(ln_w, np.float32)
    ln_b = np.asarray(ln_b, np.float32)
    wu2 = np.asarray(wu_w, np.float32) * ln_w[None, :]
    wv2 = np.asarray(wv_w, np.float32) * ln_w[None, :]
    wub2 = np.asarray(wu_b, np.float32) + np.asarray(wu_w, np.float32) @ ln_b
    wvb2 = np.asarray(wv_b, np.float32) + np.asarray(wv_w, np.float32) @ ln_b

    wt = np.concatenate([wu2.T, wv2.T], axis=1)  # [512, 128]
    wt_uv = np.ascontiguousarray(wt.reshape(4, 128, 128).transpose(1, 0, 2))
    uvb = np.concatenate([wub2, wvb2])[:, None].astype(np.float32)

    wb_rep = np.ascontiguousarray(
        np.tile(np.asarray(wb_w, np.float32).T, (1, IB)))          # [64, 468]
    bb_rep = np.ascontiguousarray(
        np.tile(np.asarray(wb_b, np.float32), IB))[None, :].astype(bf)
    krow_row = np.tile(np.arange(K, dtype=np.float32), IB)[None, :]
    ident = np.eye(128, dtype=np.float32)
    return {
        "wt_uv": wt_uv, "uvb": uvb, "wb_rep": wb_rep, "bb_rep": bb_rep,
        "krow_row": krow_row, "ident": ident,
    }


def kernel(h_res, x_true, token_pad_mask, ln_w, ln_b, wu_w, wu_b, wv_w, wv_b,
           wb_w, wb_b):
    mask_np = np.asarray(token_pad_mask, np.float32)
    with_poison = not bool(np.all(mask_np == 1.0))
    nc = _get_program(with_poison)
    shared = _shared_inputs(ln_w, ln_b, wu_w, wu_b, wv_w, wv_b, wb_w, wb_b)
    in_maps = [
        _prep_core_inputs(c, h_res, x_true, mask_np, shared, with_poison)
        for c in range(NCORES)
    ]
    res = run_bass_kernel_spmd(nc, in_maps, core_ids=list(range(NCORES)))
    return _host_finish(res.results, mask_np)


# revision 17
# speedup vs baseline: 1.3945x; 1.0197x over previous
"""DistogramLoss Trainium2 kernel (8-core SPMD, bass/tile).

Sharding: rows of the (b, i) pair-grid axis. Core c owns b = c//4 and
i in [192*(c%4), 192*(c%4)+192). The host rotates the j axis by -i0 so
the core's i-rows are always rows 0..192 of its inputs (the program is
SPMD-shared; j-reductions are order-invariant).

Layout: partitions = j (128 per block), free = (i, k) with 12 i's and
K=39 bins per supertile (free dim 468).
  L[j, 39*i+k] = sum_c V[j,c] * (wb[k,c]*U[i,c]) + bb[k]   (PE, bf16)
  ce = ln(sum_k exp(L)) - L[t]
Per supertile: one bf16 matmul with a 65th ones*bb row for the bias;
ACT exp (bf16 out); DVE grouped-reduce for sum_k exp; one-hot mask via
GPSIMD broadcast-copy + DVE bf16 is_equal; sum of L[target] via
scalar_tensor_tensor(mask*L) accum_out into a per-supertile column.
All ln's are batched into a single end-of-kernel ACT op (avoids
per-supertile activation-table reloads).
"""

import os
import sys

for _p in ("/opt/trn_rl_repo", "/opt/pypackages"):
    if os.path.isdir(_p) and _p not in sys.path:
        sys.path.append(_p)

import numpy as np

import concourse.bacc as bacc
import concourse.bass as bass
import concourse.tile as tile
from concourse import mybir
from concourse.bass_utils import run_bass_kernel_spmd

F32 = mybir.dt.float32
BF16 = mybir.dt.bfloat16
AX = mybir.AxisListType
ALU = mybir.AluOpType
ACTF = mybir.ActivationFunctionType

B, N, D, DL, K = 2, 768, 512, 64, 39
DIST_MIN, DIST_MAX = 2.0, 22.0
W = (DIST_MAX - DIST_MIN) / (K - 1)
LN_EPS = 1e-5

NCORES = 8
NI = (B * N) // NCORES          # 192 i-rows per core
IB = 12                          # i's per supertile
NIB = NI // IB                   # 16 supertiles along i
JB = 128                         # j's per block (partitions)
NJB = N // JB                    # 6 j blocks
FD = IB * K                      # 468 free dim of a supertile
NST = NJB * NIB                  # 96 supertiles
POISON = 3.0 * K                 # target offset that can never match k


def _bcast_free(ap, reps):
    """Append a 0-step dim of size `reps` to an AP (free-dim broadcast)."""
    return bass.AP(tensor=ap.tensor, offset=ap.offset, ap=list(ap.ap) + [[0, reps]])


def _build_program(with_poison: bool):
    nc = bacc.Bacc("TRN2", target_bir_lowering=False, debug=False)

    h_rows = nc.dram_tensor("h_rows", [N, D], F32, kind="ExternalInput")
    dl5 = nc.dram_tensor("dl5", [5, N], F32, kind="ExternalInput")
    dr5 = nc.dram_tensor("dr5", [5, NI], F32, kind="ExternalInput")
    wt_uv = nc.dram_tensor("wt_uv", [128, 4, 128], F32, kind="ExternalInput")
    uvb = nc.dram_tensor("uvb", [128, 1], F32, kind="ExternalInput")
    wb_rep = nc.dram_tensor("wb_rep", [DL, FD], F32, kind="ExternalInput")
    bb_rep = nc.dram_tensor("bb_rep", [1, FD], BF16, kind="ExternalInput")
    krow_row = nc.dram_tensor("krow_row", [1, FD], F32, kind="ExternalInput")
    mj_cols = nc.dram_tensor("mj_cols", [JB, NJB], F32, kind="ExternalInput")
    ident = nc.dram_tensor("ident", [128, 128], F32, kind="ExternalInput")
    if with_poison:
        poisj_cols = nc.dram_tensor("poisj_cols", [JB, NJB], F32, kind="ExternalInput")
        pois_i = nc.dram_tensor("pois_i", [1, NI], F32, kind="ExternalInput")

    out_lse = nc.dram_tensor("out_lse", [JB, NI], F32, kind="ExternalOutput")
    out_ext = nc.dram_tensor("out_ext", [JB, NST], F32, kind="ExternalOutput")

    with tile.TileContext(nc) as tc:
        with (
            tc.tile_pool(name="const", bufs=1) as const,
            tc.tile_pool(name="work", bufs=3) as work,
            tc.tile_pool(name="small", bufs=6) as small,
            tc.tile_pool(name="ebuf", bufs=4) as ebuf,
            tc.tile_pool(name="mbuf", bufs=4) as mbuf,
            tc.tile_pool(name="tbuf", bufs=3) as tbuf,
            tc.tile_pool(name="jbuf", bufs=2) as jbuf,
            tc.tile_pool(name="pp", bufs=2, space="PSUM") as pp,
            tc.tile_pool(name="psl", bufs=5, space="PSUM") as psl,
        ):
            # ---------------- constants into SBUF ----------------
            sb_wtuv = const.tile([128, 4, 128], F32)
            nc.sync.dma_start(out=sb_wtuv[:], in_=wt_uv[:])
            sb_uvb = const.tile([128, 1], F32)
            nc.sync.dma_start(out=sb_uvb[:], in_=uvb[:])
            sb_wbrep = const.tile([DL, FD], F32)
            nc.sync.dma_start(out=sb_wbrep[:], in_=wb_rep[:])
            sb_dl = const.tile([5, N], F32)
            nc.sync.dma_start(out=sb_dl[:], in_=dl5[:])
            sb_dr = const.tile([5, NI], F32)
            nc.sync.dma_start(out=sb_dr[:], in_=dr5[:])
            sb_mj = const.tile([JB, NJB], F32)
            nc.sync.dma_start(out=sb_mj[:], in_=mj_cols[:])
            sb_ident = const.tile([128, 128], F32)
            nc.sync.dma_start(out=sb_ident[:], in_=ident[:])
            sb_krow2 = const.tile([128, 2 * FD], F32)
            nc.sync.dma_start(
                out=sb_krow2[:].rearrange("p (h f) -> p h f", f=FD),
                in_=bass.AP(tensor=krow_row, offset=0,
                            ap=[[0, 128], [0, 2], [1, FD]]),
            )
            if with_poison:
                sb_poisj = const.tile([JB, NJB], F32)
                nc.sync.dma_start(out=sb_poisj[:], in_=poisj_cols[:])
                sb_poisi = const.tile([1, NI], F32)
                nc.sync.dma_start(out=sb_poisi[:], in_=pois_i[:])

            sb_eps = const.tile([128, 1], F32)
            nc.vector.memset(sb_eps[:], LN_EPS)

            s_all = const.tile([JB, NJB, NI], F32)      # sum_k exp, per (jb, i)
            ext_all = const.tile([JB, NST], F32)        # sum mask*L per supertile
            acc_lse = const.tile([JB, NI], F32)
            nc.vector.memset(acc_lse[:], 0.0)

            # ---------------- LN + transpose + projections ----------------
            hT = const.tile([128, 4, N], F32)  # h^T, c-chunk q on partitions
            for blk in range(NJB):
                hb = work.tile([128, D], F32, tag="hb")
                nc.sync.dma_start(out=hb[:], in_=h_rows[blk * 128:(blk + 1) * 128, :])
                stats = small.tile([128, 6], F32, tag="stats")
                nc.vector.bn_stats(out=stats[:], in_=hb[:])
                mv = small.tile([128, 2], F32, tag="mv")
                nc.vector.bn_aggr(out=mv[:], in_=stats[:])
                std = small.tile([128, 1], F32, tag="std")
                nc.scalar.activation(std[:], mv[:, 1:2], ACTF.Sqrt, bias=sb_eps[:, 0:1])
                rstd = small.tile([128, 1], F32, tag="rstd")
                nc.vector.reciprocal(rstd[:], std[:])
                nb = small.tile([128, 1], F32, tag="nb")
                nc.vector.tensor_scalar(
                    out=nb[:], in0=mv[:, 0:1], scalar1=rstd[:, 0:1], scalar2=-1.0,
                    op0=ALU.mult, op1=ALU.mult,
                )
                hn = work.tile([128, D], F32, tag="hn")
                nc.scalar.activation(
                    hn[:], hb[:], ACTF.Identity, bias=nb[:, 0:1], scale=rstd[:, 0:1],
                )
                for q in range(4):
                    pt = pp.tile([128, 128], F32, tag="pp")
                    nc.tensor.transpose(pt[:], hn[:, q * 128:(q + 1) * 128], sb_ident[:])
                    nc.scalar.copy(hT[:, q, blk * 128:(blk + 1) * 128], pt[:])

            uv = const.tile([128, N], F32)  # rows 0:64 U^T, 64:128 V^T
            for half in range(2):
                pu = pp.tile([128, N // 2], F32, tag="pp")
                for q in range(4):
                    nc.tensor.matmul(
                        out=pu[:], lhsT=sb_wtuv[:, q, :],
                        rhs=hT[:, q, half * (N // 2):(half + 1) * (N // 2)],
                        start=(q == 0), stop=(q == 3),
                    )
                nc.scalar.activation(
                    uv[:, half * (N // 2):(half + 1) * (N // 2)], pu[:],
                    ACTF.Identity, bias=sb_uvb[:, 0:1],
                )
            # V^T (bf16) at partitions 0:64 + ones row 64 (pairs with bb row)
            vtf = const.tile([DL, N], F32)
            nc.sync.dma_start(out=vtf[:], in_=uv[DL:128, :])
            vt65 = const.tile([DL + 1, N], BF16)
            nc.vector.tensor_copy(vt65[0:DL, :], vtf[:])
            nc.vector.memset(vt65[DL:DL + 1, :], 1.0)

            # ---------------- targets T[j, i] per j-block (bf16) ----------
            t_all = const.tile([128, NJB, NI], F32)
            for jb in range(NJB):
                pd = pp.tile([128, NI], F32, tag="pp")
                nc.tensor.matmul(
                    out=pd[:], lhsT=sb_dl[:, jb * 128:(jb + 1) * 128], rhs=sb_dr[:],
                    start=True, stop=True,
                )
                dsq = work.tile([128, NI], F32, tag="dsq")
                nc.vector.tensor_scalar(
                    out=dsq[:], in0=pd[:], scalar1=0.0, scalar2=None, op0=ALU.max,
                )
                yv = work.tile([128, NI], F32, tag="yv")  # sqrt(dsq)/W
                nc.scalar.activation(yv[:], dsq[:], ACTF.Sqrt, scale=1.0 / (W * W))
                y = work.tile([128, NI], F32, tag="y")  # (d - 2)/W
                nc.vector.tensor_scalar(
                    out=y[:], in0=yv[:], scalar1=DIST_MIN / W, scalar2=None,
                    op0=ALU.subtract,
                )
                ti = work.tile([128, NI], mybir.dt.int32, tag="ti")
                nc.vector.tensor_copy(ti[:], y[:])
                tf = work.tile([128, NI], F32, tag="tf")
                nc.vector.tensor_copy(tf[:], ti[:])
                gt = work.tile([128, NI], F32, tag="gt")
                nc.vector.tensor_tensor(out=gt[:], in0=tf[:], in1=y[:], op=ALU.is_gt)
                t0 = work.tile([128, NI], F32, tag="t0")
                nc.vector.tensor_tensor(out=t0[:], in0=tf[:], in1=gt[:], op=ALU.subtract)
                if with_poison:
                    t1 = work.tile([128, NI], F32, tag="t1")
                    nc.vector.tensor_scalar(
                        out=t1[:], in0=t0[:], scalar1=0.0, scalar2=float(K - 1),
                        op0=ALU.max, op1=ALU.min,
                    )
                    t2 = work.tile([128, NI], F32, tag="t2")
                    nc.vector.tensor_scalar(
                        out=t2[:], in0=t1[:], scalar1=sb_poisj[:, jb:jb + 1],
                        scalar2=None, op0=ALU.add,
                    )
                    pi = pp.tile([128, NI], F32, tag="pp")
                    oner = small.tile([1, 128], F32, tag="oner")
                    nc.vector.memset(oner[:], 1.0)
                    nc.tensor.matmul(
                        out=pi[:], lhsT=oner[:], rhs=sb_poisi[:],
                        start=True, stop=True,
                    )
                    nc.vector.tensor_tensor(
                        out=t_all[:, jb, :], in0=t2[:], in1=pi[:], op=ALU.add,
                    )
                else:
                    nc.vector.tensor_scalar(
                        out=t_all[:, jb, :], in0=t0[:], scalar1=0.0,
                        scalar2=float(K - 1), op0=ALU.max, op1=ALU.min,
                    )

            # -------- WU65[c, (i,k)] = wb[k,c]*U[i,c]; row 64 = bb ---------
            wu65 = const.tile([DL + 1, NIB, FD], BF16)
            wb3 = sb_wbrep[:].rearrange("p (i k) -> p i k", k=K)
            for ib in range(NIB):
                u_sl = uv[0:DL, ib * IB:(ib + 1) * IB]
                nc.vector.tensor_tensor(
                    out=wu65[0:DL, ib, :].rearrange("p (i k) -> p i k", k=K),
                    in0=wb3, in1=_bcast_free(u_sl, K), op=ALU.mult,
                )
            nc.sync.dma_start(
                out=wu65[DL:DL + 1, :, :],
                in_=bass.AP(tensor=bb_rep, offset=0, ap=[[0, 1], [0, NIB], [1, FD]]),
            )

            # ---------------- main loop (pairs of supertiles) -------------
            for jb in range(NJB):
                for ib0 in range(0, NIB, 2):
                    pls = []
                    e2 = ebuf.tile([128, 2, FD], BF16, tag="e")
                    for h in range(2):
                        ib = ib0 + h
                        pl = psl.tile([128, FD], F32, tag="psl")
                        nc.tensor.matmul(
                            out=pl[:], lhsT=vt65[:, jb * 128:(jb + 1) * 128],
                            rhs=wu65[:, ib, :], start=True, stop=True,
                        )
                        nc.scalar.activation(e2[:, h, :], pl[:], ACTF.Exp)
                        pls.append(pl)
                    nc.vector.reduce_sum(
                        out=s_all[:, jb, ib0 * IB:(ib0 + 2) * IB],
                        in_=e2[:].rearrange("p h (i k) -> p (h i) k", k=K),
                        axis=AX.X,
                    )
                    t_sl = t_all[:, jb, ib0 * IB:(ib0 + 2) * IB]
                    msk2 = mbuf.tile([128, 2 * FD], BF16, tag="msk")
                    nc.vector.tensor_tensor(
                        out=msk2[:].rearrange("p (i k) -> p i k", k=K),
                        in0=_bcast_free(t_sl, K),
                        in1=sb_krow2[:].rearrange("p (i k) -> p i k", k=K),
                        op=ALU.is_equal,
                    )
                    for h in range(2):
                        st = jb * NIB + ib0 + h
                        junk = jbuf.tile([128, FD], BF16, tag="junk")
                        nc.vector.scalar_tensor_tensor(
                            out=junk[:], in0=msk2[:, h * FD:(h + 1) * FD],
                            scalar=1.0, in1=pls[h][:],
                            op0=ALU.mult, op1=ALU.mult,
                            accum_out=ext_all[:, st:st + 1],
                        )

            # ---------------- epilogue: batched ln + masked sums ----------
            lse_all = const.tile([JB, NJB, NI], F32)
            nc.scalar.activation(lse_all[:], s_all[:], ACTF.Ln)
            for jb in range(NJB):
                nc.vector.scalar_tensor_tensor(
                    out=acc_lse[:], in0=lse_all[:, jb, :],
                    scalar=sb_mj[:, jb:jb + 1], in1=acc_lse[:],
                    op0=ALU.mult, op1=ALU.add,
                )

            nc.sync.dma_start(out=out_lse[:], in_=acc_lse[:])
            nc.sync.dma_start(out=out_ext[:], in_=ext_all[:])

    nc.finalize()
    return nc


_PROGRAM_CACHE: dict = {}


def _get_program(with_poison: bool):
    if with_poison not in _PROGRAM_CACHE:
        _PROGRAM_CACHE[with_poison] = _build_program(with_poison)
    return _PROGRAM_CACHE[with_poison]


def _prep_core_inputs(core, h_res, x_true, token_pad_mask, shared, with_poison):
    # The device program is SPMD-shared, so the U-projection always reads
    # rows 0..NI. Rotate the whole j-axis by -i0 on the host so the core's
    # i-slice lands at rows 0..NI; every j-reduction is order-invariant.
    b = core // (NCORES // B)
    i0 = NI * (core % (NCORES // B))
    x = np.roll(np.asarray(x_true[b], np.float32), -i0, axis=0)      # [N, 3]
    n2 = (x * x).sum(-1).astype(np.float32)                          # [N]
    m = np.roll(np.asarray(token_pad_mask[b], np.float32), -i0)      # [N]

    dl = np.empty((5, N), np.float32)
    dl[0:3] = -2.0 * x.T
    dl[3] = 1.0
    dl[4] = n2
    dr = np.empty((5, NI), np.float32)
    dr[0:3] = x.T[:, :NI]
    dr[3] = n2[:NI]
    dr[4] = 1.0

    inp = dict(shared)
    inp["h_rows"] = np.ascontiguousarray(
        np.roll(np.asarray(h_res[b], np.float32), -i0, axis=0))
    inp["dl5"] = dl
    inp["dr5"] = dr
    inp["mj_cols"] = np.ascontiguousarray(m.reshape(NJB, JB).T)
    if with_poison:
        inp["poisj_cols"] = np.ascontiguousarray(
            (POISON * (1.0 - m)).reshape(NJB, JB).T.astype(np.float32))
        inp["pois_i"] = (POISON * (1.0 - m[:NI]))[None, :].astype(np.float32)
    return inp


def _host_finish(results, token_pad_mask):
    mask = np.asarray(token_pad_mask, np.float64)
    ce_b = np.zeros(B, np.float64)
    per_b = NCORES // B
    for core, res in enumerate(results):
        b = core // per_b
        i0 = NI * (core % per_b)
        m_i = mask[b, i0:i0 + NI]
        lse_i = np.asarray(res["out_lse"], np.float64).sum(axis=0)  # [NI]
        ce_b[b] += float((m_i * lse_i).sum()) - float(
            np.asarray(res["out_ext"], np.float64).sum())
    counts = mask.sum(axis=1) ** 2
    per_sample = ce_b / np.maximum(counts, 1.0)
    valid = counts > 0
    total = max(float(valid.sum()), 1.0)
    loss = float(np.where(valid, per_sample, 0.0).sum() / total)
    return np.float32(loss)


def _shared_inputs(ln_w, ln_b, wu_w, wu_b, wv_w, wv_b, wb_w, wb_b):
    import ml_dtypes
    bf = ml_dtypes.bfloat16
    ln_w = np.asarray(ln_w, np.float32)
    ln_b = np.asarray(ln_b, np.float32)
    wu2 = np.asarray(wu_w, np.float32) * ln_w[None, :]
    wv2 = np.asarray(wv_w, np.float32) * ln_w[None, :]
    wub2 = np.asarray(wu_b, np.float32) + np.asarray(wu_w, np.float32) @ ln_b
    wvb2 = np.asarray(wv_b, np.float32) + np.asarray(wv_w, np.float32) @ ln_b

    wt = np.concatenate([wu2.T, wv2.T], axis=1)  # [512, 128]
    wt_uv = np.ascontiguousarray(wt.reshape(4, 128, 128).transpose(1, 0, 2))
    uvb = np.concatenate([wub2, wvb2])[:, None].astype(np.float32)

    wb_rep = np.ascontiguousarray(
        np.tile(np.asarray(wb_w, np.float32).T, (1, IB)))          # [64, 468]
    bb_rep = np.ascontiguousarray(
        np.tile(np.asarray(wb_b, np.float32), IB))[None, :].astype(bf)
    krow_row = np.tile(np.arange(K, dtype=np.float32), IB)[None, :]
    ident = np.eye(128, dtype=np.float32)
    return {
        "wt_uv": wt_uv, "uvb": uvb, "wb_rep": wb_rep, "bb_rep": bb_rep,
        "krow_row": krow_row, "ident": ident,
    }


def kernel(h_res, x_true, token_pad_mask, ln_w, ln_b, wu_w, wu_b, wv_w, wv_b,
           wb_w, wb_b):
    mask_np = np.asarray(token_pad_mask, np.float32)
    with_poison = not bool(np.all(mask_np == 1.0))
    nc = _get_program(with_poison)
    shared = _shared_inputs(ln_w, ln_b, wu_w, wu_b, wv_w, wv_b, wb_w, wb_b)
    in_maps = [
        _prep_core_inputs(c, h_res, x_true, mask_np, shared, with_poison)
        for c in range(NCORES)
    ]
    res = run_bass_kernel_spmd(nc, in_maps, core_ids=list(range(NCORES)))
    return _host_finish(res.results, mask_np)
